# revision 39
# baseline (speedup 1.0000x reference)
"""NetVLAD pooling kernel for Trainium2, data-parallel over batch across 8 cores.

Computation per batch b (reference semantics):
  y      = x @ W_red.T + b_red            # [m, 64]
  yn     = y / ||y||_row                  # L2 normalize rows
  logits = yn @ W_lin.T + b_lin           # [m, 8]
  a      = softmax(logits, axis=1)
  vlad   = a.T @ yn - centroids * a.sum(0)[:, None]
  out    = l2norm_global(l2norm_rows(vlad).flatten())

Device-side algebra (per row m):
  yz   = x @ [W_red.T | W_red.T W_lin.T] + [b_red | W_lin b_red]   # fused [m, 72]
  inv  = exp(-0.5 ln(sum(y^2)))        # 1/||y|| via the ln/exp table set
  n    = ss * inv                      # ||y||
  e    = exp(raw2 * inv)               # un-biased softmax numerator
  r    = 1 / sum_k(e * exp(b_lin))
  atil = e * exp(b_lin) * (inv * r)    # atil.T @ [y | n] = ebl*[a'.T yn | a'.sum]
  so the per-batch accumulator arrives pre-scaled by exp(b_lin) and the
  finalize is just centroid-subtract + intra/global normalization.

Layout and schedule:
- x ships pre-transposed to [b, C, m] in fp8e4m3: the contraction dim lands
  on SBUF partitions with 1KB-contiguous DMA descriptors at 1 byte/elem
  (the kernel is HBM-bound: 16.8 MB/core floor).
- The reduction matmul runs in fp8 DoubleRow mode: two 128-row contraction
  chunks per instruction at half per-row cost (4x fewer PE cycles than
  four plain chunk matmuls).
- Work is organized in 2048-row supersteps, software-pipelined in 6 stages
  (mm+ycopy | square+reduce | ln/exp + t64/n | exp + ebl | rsum | atil) with
  the aggregation matmuls 5 supersteps behind, so every cross-engine
  dependency is about a superstep old when its in-order consumer reaches it.
- Engine placement keeps each engine under the 2.9us/superstep DMA pace:
  ACT does the PSUM->SBUF bf16 y copy (GPSIMD cannot touch PSUM) plus
  Ln/Exp/softmax-Exp; DVE does the square (2x bf16 rate), the big reduce
  and r; Pool (GpSimd) does the inv scalings and exp(b_lin) weighting.
- Each superstep's aggregation closes its own PSUM group, folded into an
  SBUF accumulator, so no PSUM bank is held across the deferral window
  (PSUM: 3x2 yz banks + 2 vladpart banks = 8).
- xt loads go out in 2-tile groups, rotated across the SP and ACT hardware
  DGE queues ~14 tiles ahead of consumption (the issuing sequencer is held
  for descriptor-gen plus wire wait, so one queue alone cannot keep the
  wire saturated).
- The output is centroid-dominated (||a.T yn|| ~ 21 vs ||cent * asum|| ~
  4700), so fp8 x / bf16 y quantization lands ~3e-4 relative to output
  scale, far under the 2e-2 gate.
"""
import numpy as np
import ml_dtypes
from contextlib import ExitStack

import concourse.bass as bass
import concourse.tile as tile
import concourse.bass_isa as bass_isa
from concourse import bacc, mybir
from concourse._compat import with_exitstack
from concourse.bass_utils import run_bass_kernel_spmd

bf16 = ml_dtypes.bfloat16
F32 = mybir.dt.float32
BF16 = mybir.dt.bfloat16
FP8 = mybir.dt.float8e4
fp8 = ml_dtypes.float8_e4m3

# tuning toggles (read at program-build time)
SB_BUFS = 8
XT_BUFS = 9          # xt group buffers (each up to GROUP m-tiles)
DEFER = 5            # supersteps between a superstep's matmul and its agg
PREFETCH_T = 14      # m-tiles of xt DMA lookahead
GROUP = 2            # m-tiles per xt DMA (amortizes per-copy queue overhead)
DMA_QUEUES = ("sync", "scalar")   # rotate xt groups across HWDGE queues

N_CORES = 8
B, M, C = 32, 8192, 512
K, D = 8, 64
B_LOC = B // N_CORES          # 4 batches per core
M_TILE = 1024
N_TILES = M // M_TILE         # 8
SUB = M_TILE // 128           # 8 subtiles of 128 rows
NCH = C // 128                # 4 contraction chunks


@with_exitstack
def _netvlad_kernel(ctx: ExitStack, tc: tile.TileContext, out_d, xt_d, wcat_d,
                    bcat_d, eblbc_d, ebl8_d, cent_d):
    nc = tc.nc
    AF = mybir.ActivationFunctionType
    OP = mybir.AluOpType
    PM = mybir.MatmulPerfMode

    consts = ctx.enter_context(tc.tile_pool(name="consts", bufs=1))
    xt_pool = ctx.enter_context(tc.tile_pool(name="xt", bufs=XT_BUFS))
    sb = ctx.enter_context(tc.tile_pool(name="work", bufs=SB_BUFS))
    vaccp = ctx.enter_context(tc.tile_pool(name="vaccp", bufs=2))
    outp = ctx.enter_context(tc.tile_pool(name="outp", bufs=1))
    yz_pool = ctx.enter_context(tc.tile_pool(name="yz", bufs=3, space="PSUM"))
    vp_pool = ctx.enter_context(tc.tile_pool(name="vp", bufs=2, space="PSUM"))

    # constants: wcat/bcat are needed by the very first matmuls; the rest
    # are deferred into the xt group stream (emitted by the prefetch pump)
    # so their per-copy queue overhead doesn't delay the first x tiles
    wcat = consts.tile([128, NCH, 72], FP8)
    bcat = consts.tile([1, 72], FP8)
    eblbc = consts.tile([128, 2 * SUB, K], F32)
    cent = consts.tile([K, D], F32)
    ones = consts.tile([1, 128], FP8)
    nc.vector.memset(ones[:], 1.0)

    def emit_consts_early():
        nc.sync.dma_start(wcat[:], wcat_d.rearrange("j p t -> p j t"))
        nc.sync.dma_start(bcat[:], bcat_d[:])

    def emit_consts_mid():
        nc.scalar.dma_start(eblbc[:], eblbc_d[:])

    def emit_consts_late():
        nc.scalar.dma_start(cent[:], cent_d[:])

    outsb = outp.tile([K, B_LOC, D], F32)

    dma_idx = [0]

    def emit_dma(xt_b, m0, rows):
        """Prefetch a group of m-tiles of x on a rotating HWDGE queue."""
        xt = xt_pool.tile([128, NCH, rows], FP8, tag="xt", name="xt")
        q = getattr(nc, DMA_QUEUES[dma_idx[0] % len(DMA_QUEUES)])
        dma_idx[0] += 1
        q.dma_start(xt[:], xt_b[:, :, m0:m0 + rows])
        return xt

    # Each superstep covers up to two 1024-row m-tiles (2048 descriptors) so
    # per-instruction overheads amortize over twice the data.  The chain is
    # software-pipelined: stage Ak of superstep i is emitted at step i+k, so
    # every cross-engine dependency is a superstep (~2.9us of work) old when
    # the consumer's in-order queue reaches it.  Per-superstep engine
    # budgets, against the ~2.9us DMA wire pace:
    #   ACT  2x ycopy 665 + Ln 198 + Exp 198 + E64 292   ~ 2.0us
    #   DVE  sq 593 + red 1127 + rs8 193 + rcp 77
    #        + q8 77 + atil 254 + vacc 193               ~ 2.5us
    #   Pool t64 444 + n 250 + am 349                    ~ 1.0us
    #   PE   48 mm + 16 agg mm                           ~ 1.4us
    # The y copy carries all 72 yz columns to SBUF in bf16 (y | raw2); the
    # squares come from its bf16 y at the DVE 2x rate, raw2 is consumed in
    # bf16, and column 64 is overwritten with n = ||y|| (same in-order Pool
    # queue as its reader) to form the agg rhs [y | n].  Each superstep's
    # agg matmuls close their own PSUM accumulation group (vladpart) which
    # DVE folds into an SBUF accumulator, so no PSUM bank is held across the
    # deferral window.  GPSIMD cannot touch PSUM, so PSUM reads sit on
    # ACT/DVE only.
    def a0_matmul(st):
        # fused reduction+logits matmul: yz[m, :72] = x @ Wcat + bcat
        # (fp8 DoubleRow: two 128-row chunks per instruction, half per-row
        # cost)
        W = st["w"]
        yzs = []
        for t in range(W):
            xt = st["xts"][t]
            yz = yz_pool.tile([128, SUB, 128], F32, tag="yz", name="yz")
            for s in range(SUB):
                sc = slice(s * 128, (s + 1) * 128)
                nc.tensor.matmul(yz[:, s, :72], xt[:, 0:2, sc],
                                 wcat[:, 0:2, :], start=True, stop=False,
                                 perf_mode=PM.DoubleRow)
                nc.tensor.matmul(yz[:, s, :72], xt[:, 2:4, sc],
                                 wcat[:, 2:4, :], start=False, stop=False,
                                 perf_mode=PM.DoubleRow)
                nc.tensor.matmul(yz[:, s, :72], ones[:], bcat[:],
                                 start=False, stop=True)
            yzs.append(yz)
        st["yzs"] = yzs
        del st["xts"]

    def a0_ycopy(st):
        # PSUM -> SBUF in bf16 on ACT (GPSIMD cannot touch PSUM)
        W = st["w"]
        rb = sb.tile([128, W * SUB, 72], BF16, tag="rb", name="rb")
        for t, yz in enumerate(st["yzs"]):
            nc.scalar.activation(rb[:, t * SUB:(t + 1) * SUB, :],
                                 yz[:, :, :72], AF.Copy)
        st["rb"] = rb
        del st["yzs"]

    def a1_norm2(st):
        # ss = sum(y^2) per row, from the bf16 y at the DVE 2x rate
        n = st["w"] * SUB
        sqs = sb.tile([128, n, D], BF16, tag="sqs", name="sqs")
        nc.vector.tensor_tensor(out=sqs[:], in0=st["rb"][:, :, :D],
                                in1=st["rb"][:, :, :D], op=OP.mult)
        ss8 = sb.tile([128, n], F32, tag="ss8", name="ss8")
        nc.vector.reduce_sum(ss8[:], sqs[:], axis=mybir.AxisListType.X)
        st["ss8"] = ss8

    def a2_inv(st):
        # inv = 1/||y|| via the ln/exp table set; Pool consumes it in-step
        # (its queue reaches t64 after this ACT pair has retired)
        n = st["w"] * SUB
        lss = sb.tile([128, n], F32, tag="lss", name="lss")
        nc.scalar.activation(lss[:], st["ss8"][:], AF.Ln)
        inv8 = sb.tile([128, n], F32, tag="inv8", name="inv8")
        nc.scalar.activation(inv8[:], lss[:], AF.Exp, scale=-0.5)
        t64 = sb.tile([128, n, K], F32, tag="t64", name="t64")
        nc.gpsimd.tensor_tensor(
            out=t64[:], in0=st["rb"][:, :, D:D + K],
            in1=inv8[:].unsqueeze(2).broadcast_to([128, n, K]), op=OP.mult)
        # n = ss * inv = ||y||, into rhs column 64 (after its raw2 read,
        # same in-order Pool queue)
        nc.gpsimd.tensor_tensor(out=st["rb"][:, :, D:D + 1],
                                in0=st["ss8"][:].unsqueeze(2),
                                in1=inv8[:].unsqueeze(2), op=OP.mult)
        st["inv8"], st["t64"] = inv8, t64
        del st["ss8"]

    def a3_exp(st):
        # softmax numerators e = exp(raw2 * inv), then the exp(b_lin)
        # weighting on Pool in-step (am follows E64 in emission order)
        n = st["w"] * SUB
        e64 = sb.tile([128, n, K], F32, tag="e64", name="e64")
        nc.scalar.activation(e64[:], st["t64"][:], AF.Exp)
        am = sb.tile([128, n, K], F32, tag="am", name="am")
        nc.gpsimd.tensor_tensor(out=am[:], in0=e64[:], in1=eblbc[:, :n, :],
                                op=OP.mult)
        st["am"] = am
        del st["t64"]

    def a4_rsum(st):
        # r = 1/sum_k(e * exp(b_lin))
        n = st["w"] * SUB
        rs8 = sb.tile([128, n], F32, tag="rs8", name="rs8")
        nc.vector.reduce_sum(rs8[:], st["am"][:], axis=mybir.AxisListType.X)
        rr8 = sb.tile([128, n], F32, tag="rr8", name="rr8")
        nc.vector.reciprocal(rr8[:], rs8[:])
        st["rr8"] = rr8

    def a4_atil(st):
        # atil = e * (inv * r)
        n = st["w"] * SUB
        q8 = sb.tile([128, n], F32, tag="q8", name="q8")
        nc.gpsimd.tensor_tensor(out=q8[:], in0=st["inv8"][:],
                                in1=st["rr8"][:], op=OP.mult)
        # atil = am * q = e * exp(b_lin) * inv * r, so vlad rows arrive
        # pre-scaled by exp(b_lin) and the finalize skips that multiply
        atil = sb.tile([128, n, K], BF16, tag="atil", name="atil")
        nc.gpsimd.tensor_tensor(
            out=atil[:], in0=st["am"][:],
            in1=q8[:].unsqueeze(2).broadcast_to([128, n, K]), op=OP.mult)
        st["atil"] = atil
        del st["am"], st["inv8"], st["rr8"]

    def a5_agg(st):
        # vladpart[k, :] = sum_s atil_s.T @ [y | n]; one PSUM group per
        # superstep, folded into the batch's SBUF accumulator right after
        n = st["w"] * SUB
        atil, rb = st["atil"], st["rb"]
        vp = vp_pool.tile([K, D + 1], F32, tag="vp", name="vp")
        for s in range(n):
            nc.tensor.matmul(vp[:], atil[:, s, :], rb[:, s, :D + 1],
                             start=(s == 0), stop=(s == n - 1))
        if st["first"]:
            nc.vector.tensor_copy(st["vacc"][:], vp[:])
        else:
            nc.vector.tensor_tensor(out=st["vacc"][:], in0=st["vacc"][:],
                                    in1=vp[:], op=OP.add)
        if st["last"]:
            emit_finalize(st["vacc"], st["b"])
        st.clear()

    def emit_finalize(vlad, b):  # vlad: SBUF accumulator, pre-scaled by ebl
        # finalize batch: centroid subtract, intra-normalize, global norm
        cv = sb.tile([K, D], F32, tag="cv", name="cv")
        nc.vector.tensor_scalar_mul(cv[:], cent[:], vlad[:, D:D + 1])
        v = sb.tile([K, D], F32, tag="v", name="v")
        nc.vector.tensor_sub(v[:], vlad[:, :D], cv[:])
        sck = sb.tile([K, D], F32, tag="sck", name="sck")
        nc.vector.tensor_tensor(out=sck[:], in0=v[:], in1=v[:], op=OP.mult)
        ssk = sb.tile([K, 1], F32, tag="ssk", name="ssk")
        nc.vector.reduce_sum(ssk[:], sck[:], axis=mybir.AxisListType.X)
        lk = sb.tile([K, 1], F32, tag="lk", name="lk")
        nc.scalar.activation(lk[:], ssk[:], AF.Ln)
        invk = sb.tile([K, 1], F32, tag="invk", name="invk")
        nc.scalar.activation(invk[:], lk[:], AF.Exp, scale=-0.5)
        # after intra-normalization each of the K rows has norm exactly 1,
        # so the global norm is sqrt(K) (fp error ~1e-7, far under budget);
        # fold 1/sqrt(K) into the intra-norm multiply
        nc.vector.tensor_scalar(
            out=outsb[:, b, :], in0=v[:], scalar1=invk[:],
            scalar2=float(1.0 / np.sqrt(K)), op0=OP.mult, op1=OP.mult)

    # xt DMA groups (in m-tiles): small leading groups fill the pipeline,
    # then GROUP-sized copies amortize the per-copy queue-hold overhead.
    # supersteps: (batch, [tile indices]) with 1-tile steps early on.
    groups = []       # (batch, m0_tiles, n_tiles)
    tile_of = {}      # global tile idx -> (group, offset)
    supers = []       # (batch, [global tile idx], first, last)
    def chunks(total, first, size):
        out = list(first)
        left = total - sum(first)
        while left > 0:
            c = min(size, left)
            out.append(c)
            left -= c
        return out

    gsizes0 = chunks(N_TILES, [1, 1], GROUP)
    ssizes0 = [1, 1, 2, 2, 2]
    gsizes = chunks(N_TILES, [], GROUP)
    ssizes = [2] * (N_TILES // 2)
    ssizesN = [2] * (N_TILES // 2)
    gtile = 0
    for b in range(B_LOC):
        t0 = 0
        for gsz in (gsizes0 if b == 0 else gsizes):
            g = len(groups)
            groups.append((b, t0, gsz))
            for j in range(gsz):
                tile_of[gtile + t0 + j] = (g, j)
            t0 += gsz
        assert t0 == N_TILES
        t0 = 0
        ss = ssizes0 if b == 0 else (ssizesN if b == B_LOC - 1 else ssizes)
        for ssz in ss:
            supers.append((b, [gtile + t0 + j for j in range(ssz)],
                           t0 == 0, t0 + ssz == N_TILES))
            t0 += ssz
        assert t0 == N_TILES
        gtile += N_TILES

    xt_bs = [xt_d[b].rearrange("(j p) m -> p j m", p=128) for b in range(B_LOC)]
    xtg = {}

    def emit_group(g):
        gb, gt0, gn = groups[g]
        xtg[g] = emit_dma(xt_bs[gb], gt0 * M_TILE, gn * M_TILE)
        if g == 3:
            emit_consts_mid()
        elif g == 5:
            emit_consts_late()

    def xt_slice(ti):
        g, off = tile_of[ti]
        return xtg[g][:, :, off * M_TILE:(off + 1) * M_TILE]

    emit_consts_early()
    lead0 = 0
    t_acc = 0
    while lead0 < len(groups) and t_acc < PREFETCH_T:
        t_acc += groups[lead0][2]
        lead0 += 1
    for g in range(lead0):
        emit_group(g)
    next_g = [lead0]
    states = {}
    vaccs = {}
    n_sup = len(supers)
    emitted_tiles = 0
    for i in range(n_sup + DEFER + 1):  # DEFER == pipeline depth (5)
        if i < n_sup:
            b, tids, first, last = supers[i]
            if first:
                vaccs[b] = vaccp.tile([K, D + 1], F32, tag="vacc",
                                      name="vacc")
            # keep the DMA queues PREFETCH_G groups ahead of consumption
            emitted_tiles += len(tids)
            while next_g[0] < len(groups) and \
                    sum(groups[g][2] for g in range(next_g[0])) < \
                    emitted_tiles + PREFETCH_T:
                emit_group(next_g[0])
                next_g[0] += 1
            states[i] = {"w": len(tids), "b": b, "first": first,
                         "last": last, "vacc": vaccs[b],
                         "xts": [xt_slice(t) for t in tids]}
        # per-engine emission order tuned so no in-order queue head-blocks:
        #   PE   mm(i), agg(i-5)
        #   ACT  E64(i-3), ycopy(i), Ln/Exp(i-2)
        #   DVE  rs8/rcp(i-4), sq/red(i-1), vacc(i-5)
        #   Pool am(i-3), t64/n(i-2), q8/atil(i-4)
        if i < n_sup:
            a0_matmul(states[i])
        if 0 <= i - 3 < n_sup:
            a3_exp(states[i - 3])
        if i < n_sup:
            a0_ycopy(states[i])
        if 0 <= i - 2 < n_sup:
            a2_inv(states[i - 2])
        if 0 <= i - 4 < n_sup:
            a4_rsum(states[i - 4])
        if 0 <= i - 1 < n_sup:
            a1_norm2(states[i - 1])
        if 0 <= i - 4 < n_sup:
            a4_atil(states[i - 4])
        if i - 5 >= 0 and states.get(i - 5):
            a5_agg(states.pop(i - 5))

    nc.sync.dma_start(out_d.rearrange("b (k d) -> k b d", k=K), outsb[:])



_CACHE = {}


def _patch_act_tables():
    """Force all Exp/Ln/Square activations to resolve in the one table set
    that contains them all (natural_log_exp_and_others), so bacc's
    insert_act_table_loads emits a single hoisted LoadActFuncSet instead of
    thrashing between exp_and_others and natural_log per tile (~2.7us per
    reload).  List order/length is preserved so act_func_set_id stays a
    valid index into act_info.json."""
    import concourse.bacc as bacc_mod
    import concourse.hw_specs as hw_specs
    if _CACHE.get("act_patched"):
        return
    orig = hw_specs.get_activation_tables
    AF = mybir.ActivationFunctionType
    strip = {AF.Exp, AF.Ln, AF.Square}
    keep = "natural_log_exp_and_others"

    def patched(arch):
        tables = orig(arch)
        return {
            name: (set(fns) if name == keep else set(fns) - strip)
            for name, fns in tables.items()
        }

    bacc_mod.get_activation_tables = patched
    _CACHE["act_patched"] = True


def _declare_io(nc):
    xt_d = nc.dram_tensor("xt", [B_LOC, C, M], FP8,
                          kind="ExternalInput").ap()
    wcat_d = nc.dram_tensor("wcat", [NCH, 128, 72], FP8,
                            kind="ExternalInput").ap()
    bcat_d = nc.dram_tensor("bcat", [1, 72], FP8,
                            kind="ExternalInput").ap()
    eblbc_d = nc.dram_tensor("eblbc", [128, 2 * SUB, K], F32,
                             kind="ExternalInput").ap()
    ebl8_d = nc.dram_tensor("ebl8", [K, 1], F32, kind="ExternalInput").ap()
    cent_d = nc.dram_tensor("cent", [K, D], F32, kind="ExternalInput").ap()
    out_d = nc.dram_tensor("out", [B_LOC, K * D], F32, kind="ExternalOutput").ap()
    return out_d, xt_d, wcat_d, bcat_d, eblbc_d, ebl8_d, cent_d


def _build_program():
    if "nc" in _CACHE:
        return _CACHE["nc"]
    _patch_act_tables()
    nc = bacc.Bacc("TRN2", target_bir_lowering=False, debug=False,
                   num_devices=N_CORES)
    out_d, xt_d, wcat_d, bcat_d, eblbc_d, ebl8_d, cent_d = _declare_io(nc)

    with tile.TileContext(nc) as tc:
        _netvlad_kernel(tc, out_d, xt_d, wcat_d, bcat_d, eblbc_d, ebl8_d, cent_d)
    nc.compile()
    _CACHE["nc"] = nc
    return nc


def _prep_inputs(x, W_red, b_red, W_lin, b_lin, centroids):
    wcat = np.concatenate([W_red.T, W_red.T @ W_lin.T], axis=1)     # [512, 72]
    wcat = np.ascontiguousarray(wcat.astype(fp8).reshape(NCH, 128, 72))
    bcat = np.concatenate([b_red, W_lin @ b_red]).astype(fp8)[None, :]
    ebl = np.exp(b_lin).astype(np.float32)
    eblbc = np.ascontiguousarray(
        np.broadcast_to(ebl, (128, 2 * SUB, K)).astype(np.float32))
    ebl8 = ebl[:, None]
    cent = centroids.astype(np.float32)
    xt = np.ascontiguousarray(x.astype(fp8).transpose(0, 2, 1))     # [B, C, M]
    return xt, wcat, bcat, eblbc, ebl8, cent


def kernel(x, mask, W_red, b_red, W_lin, b_lin, centroids, **kwargs):
    x = np.asarray(x, dtype=np.float32)
    W_red = np.asarray(W_red, dtype=np.float32)
    b_red = np.asarray(b_red, dtype=np.float32)
    W_lin = np.asarray(W_lin, dtype=np.float32)
    b_lin = np.asarray(b_lin, dtype=np.float32)
    centroids = np.asarray(centroids, dtype=np.float32)

    xt, wcat, bcat, eblbc, ebl8, cent = _prep_inputs(
        x, W_red, b_red, W_lin, b_lin, centroids)

    nc = _build_program()
    in_maps = []
    for i in range(N_CORES):
        in_maps.append({
            "xt": np.ascontiguousarray(xt[i * B_LOC:(i + 1) * B_LOC]),
            "wcat": wcat, "bcat": bcat, "eblbc": eblbc,
            "ebl8": ebl8, "cent": cent,
        })
    res = run_bass_kernel_spmd(nc, in_maps, list(range(N_CORES)),
                               **kwargs.get("_run_kwargs", {}))
    out = np.concatenate([res.results[i]["out"] for i in range(N_CORES)], axis=0)
    if kwargs.get("_return_raw"):
        return out, res
    return out


# revision 41
# speedup vs baseline: 1.0207x; 1.0207x over previous
"""NetVLAD pooling kernel for Trainium2, data-parallel over batch across 8 cores.

Computation per batch b (reference semantics):
  y      = x @ W_red.T + b_red            # [m, 64]
  yn     = y / ||y||_row                  # L2 normalize rows
  logits = yn @ W_lin.T + b_lin           # [m, 8]
  a      = softmax(logits, axis=1)
  vlad   = a.T @ yn - centroids * a.sum(0)[:, None]
  out    = l2norm_global(l2norm_rows(vlad).flatten())

Device-side algebra (per row m):
  yz   = x @ [W_red.T | W_red.T W_lin.T] + [b_red | W_lin b_red]   # fused [m, 72]
  inv  = exp(-0.5 ln(sum(y^2)))        # 1/||y|| via the ln/exp table set
  n    = ss * inv                      # ||y||
  e    = exp(raw2 * inv)               # un-biased softmax numerator
  r    = 1 / sum_k(e * exp(b_lin))
  atil = e * exp(b_lin) * (inv * r)    # atil.T @ [y | n] = ebl*[a'.T yn | a'.sum]
  so the per-batch accumulator arrives pre-scaled by exp(b_lin) and the
  finalize is just centroid-subtract + intra/global normalization.

Layout and schedule:
- x ships pre-transposed to [b, C, m] in fp8e4m3: the contraction dim lands
  on SBUF partitions with 1KB-contiguous DMA descriptors at 1 byte/elem
  (the kernel is HBM-bound: 16.8 MB/core floor).
- The reduction matmul runs in fp8 DoubleRow mode: two 128-row contraction
  chunks per instruction at half per-row cost (4x fewer PE cycles than
  four plain chunk matmuls).
- Work is organized in 2048-row supersteps, software-pipelined in 6 stages
  (mm+ycopy | square+reduce | ln/exp + t64/n | exp + ebl | rsum | atil) with
  the aggregation matmuls 5 supersteps behind, so every cross-engine
  dependency is about a superstep old when its in-order consumer reaches it.
- Engine placement keeps each engine under the 2.9us/superstep DMA pace:
  ACT does the PSUM->SBUF bf16 y copy (GPSIMD cannot touch PSUM) plus
  Ln/Exp/softmax-Exp; DVE does the square (2x bf16 rate), the big reduce
  and r; Pool (GpSimd) does the inv scalings and exp(b_lin) weighting.
- Each superstep's aggregation closes its own PSUM group, folded into an
  SBUF accumulator, so no PSUM bank is held across the deferral window
  (PSUM: 3x2 yz banks + 2 vladpart banks = 8).
- xt loads go out in 2-tile groups, rotated across the SP and ACT hardware
  DGE queues ~14 tiles ahead of consumption (the issuing sequencer is held
  for descriptor-gen plus wire wait, so one queue alone cannot keep the
  wire saturated).
- The output is centroid-dominated (||a.T yn|| ~ 21 vs ||cent * asum|| ~
  4700), so fp8 x / bf16 y quantization lands ~3e-4 relative to output
  scale, far under the 2e-2 gate.
"""
import numpy as np
import ml_dtypes
from contextlib import ExitStack

import concourse.bass as bass
import concourse.tile as tile
import concourse.bass_isa as bass_isa
from concourse import bacc, mybir
from concourse._compat import with_exitstack
from concourse.bass_utils import run_bass_kernel_spmd

bf16 = ml_dtypes.bfloat16
F32 = mybir.dt.float32
BF16 = mybir.dt.bfloat16
FP8 = mybir.dt.float8e4
fp8 = ml_dtypes.float8_e4m3

# tuning toggles (read at program-build time)
SB_BUFS = 8
XT_BUFS = 9          # xt group buffers (each up to GROUP m-tiles)
DEFER = 5            # supersteps between a superstep's matmul and its agg
PREFETCH_T = 14      # m-tiles of xt DMA lookahead
GROUP = 2            # m-tiles per xt DMA (amortizes per-copy queue overhead)
DMA_QUEUES = ("sync", "scalar")   # rotate xt groups across HWDGE queues

N_CORES = 8
B, M, C = 32, 8192, 512
K, D = 8, 64
B_LOC = B // N_CORES          # 4 batches per core
M_TILE = 1024
N_TILES = M // M_TILE         # 8
SUB = M_TILE // 128           # 8 subtiles of 128 rows
NCH = C // 128                # 4 contraction chunks


@with_exitstack
def _netvlad_kernel(ctx: ExitStack, tc: tile.TileContext, out_d, xt_d, wcat_d,
                    bcat_d, eblbc_d, ebl8_d, cent_d):
    nc = tc.nc
    AF = mybir.ActivationFunctionType
    OP = mybir.AluOpType
    PM = mybir.MatmulPerfMode

    consts = ctx.enter_context(tc.tile_pool(name="consts", bufs=1))
    xt_pool = ctx.enter_context(tc.tile_pool(name="xt", bufs=XT_BUFS))
    sb = ctx.enter_context(tc.tile_pool(name="work", bufs=SB_BUFS))
    vaccp = ctx.enter_context(tc.tile_pool(name="vaccp", bufs=2))
    outp = ctx.enter_context(tc.tile_pool(name="outp", bufs=1))
    yz_pool = ctx.enter_context(tc.tile_pool(name="yz", bufs=3, space="PSUM"))
    vp_pool = ctx.enter_context(tc.tile_pool(name="vp", bufs=2, space="PSUM"))

    # constants: wcat/bcat are needed by the very first matmuls; the rest
    # are deferred into the xt group stream (emitted by the prefetch pump)
    # so their per-copy queue overhead doesn't delay the first x tiles
    wcat = consts.tile([128, NCH, 72], FP8)
    bcat = consts.tile([1, 72], FP8)
    eblbc = consts.tile([128, 2 * SUB, K], F32)
    cent = consts.tile([K, D], F32)
    ones = consts.tile([1, 128], FP8)
    nc.vector.memset(ones[:], 1.0)

    def emit_consts_early():
        nc.sync.dma_start(wcat[:], wcat_d.rearrange("j p t -> p j t"))
        nc.sync.dma_start(bcat[:], bcat_d[:])

    def emit_consts_mid():
        nc.scalar.dma_start(eblbc[:], eblbc_d[:])

    def emit_consts_late():
        nc.scalar.dma_start(cent[:], cent_d[:])

    outsb = outp.tile([K, B_LOC, D], F32)

    dma_idx = [1]   # first group on the scalar queue, parallel to wcat

    def emit_dma(xt_b, m0, rows):
        """Prefetch a group of m-tiles of x on a rotating HWDGE queue."""
        xt = xt_pool.tile([128, NCH, rows], FP8, tag="xt", name="xt")
        q = getattr(nc, DMA_QUEUES[dma_idx[0] % len(DMA_QUEUES)])
        dma_idx[0] += 1
        q.dma_start(xt[:], xt_b[:, :, m0:m0 + rows])
        return xt

    # Each superstep covers up to two 1024-row m-tiles (2048 descriptors) so
    # per-instruction overheads amortize over twice the data.  The chain is
    # software-pipelined: stage Ak of superstep i is emitted at step i+k, so
    # every cross-engine dependency is a superstep (~2.9us of work) old when
    # the consumer's in-order queue reaches it.  Per-superstep engine
    # budgets, against the ~2.9us DMA wire pace:
    #   ACT  2x ycopy 665 + Ln 198 + Exp 198 + E64 292   ~ 2.0us
    #   DVE  sq 593 + red 1127 + rs8 193 + rcp 77
    #        + q8 77 + atil 254 + vacc 193               ~ 2.5us
    #   Pool t64 444 + n 250 + am 349                    ~ 1.0us
    #   PE   48 mm + 16 agg mm                           ~ 1.4us
    # The y copy carries all 72 yz columns to SBUF in bf16 (y | raw2); the
    # squares come from its bf16 y at the DVE 2x rate, raw2 is consumed in
    # bf16, and column 64 is overwritten with n = ||y|| (same in-order Pool
    # queue as its reader) to form the agg rhs [y | n].  Each superstep's
    # agg matmuls close their own PSUM accumulation group (vladpart) which
    # DVE folds into an SBUF accumulator, so no PSUM bank is held across the
    # deferral window.  GPSIMD cannot touch PSUM, so PSUM reads sit on
    # ACT/DVE only.
    def a0_matmul(st):
        # fused reduction+logits matmul: yz[m, :72] = x @ Wcat + bcat
        # (fp8 DoubleRow: two 128-row chunks per instruction, half per-row
        # cost)
        W = st["w"]
        yzs = []
        for t in range(W):
            xt = st["xts"][t]
            yz = yz_pool.tile([128, SUB, 128], F32, tag="yz", name="yz")
            for s in range(SUB):
                sc = slice(s * 128, (s + 1) * 128)
                nc.tensor.matmul(yz[:, s, :72], xt[:, 0:2, sc],
                                 wcat[:, 0:2, :], start=True, stop=False,
                                 perf_mode=PM.DoubleRow)
                nc.tensor.matmul(yz[:, s, :72], xt[:, 2:4, sc],
                                 wcat[:, 2:4, :], start=False, stop=False,
                                 perf_mode=PM.DoubleRow)
                nc.tensor.matmul(yz[:, s, :72], ones[:], bcat[:],
                                 start=False, stop=True)
            yzs.append(yz)
        st["yzs"] = yzs
        del st["xts"]

    def a0_ycopy(st):
        # PSUM -> SBUF in bf16 on ACT (GPSIMD cannot touch PSUM)
        W = st["w"]
        rb = sb.tile([128, W * SUB, 72], BF16, tag="rb", name="rb")
        for t, yz in enumerate(st["yzs"]):
            nc.scalar.activation(rb[:, t * SUB:(t + 1) * SUB, :],
                                 yz[:, :, :72], AF.Copy)
        st["rb"] = rb
        del st["yzs"]

    def a1_norm2(st):
        # ss = sum(y^2) per row, from the bf16 y at the DVE 2x rate
        n = st["w"] * SUB
        sqs = sb.tile([128, n, D], BF16, tag="sqs", name="sqs")
        nc.vector.tensor_tensor(out=sqs[:], in0=st["rb"][:, :, :D],
                                in1=st["rb"][:, :, :D], op=OP.mult)
        ss8 = sb.tile([128, n], F32, tag="ss8", name="ss8")
        nc.vector.reduce_sum(ss8[:], sqs[:], axis=mybir.AxisListType.X)
        st["ss8"] = ss8

    def a2_inv(st):
        # inv = 1/||y|| via the ln/exp table set; Pool consumes it in-step
        # (its queue reaches t64 after this ACT pair has retired)
        n = st["w"] * SUB
        lss = sb.tile([128, n], F32, tag="lss", name="lss")
        nc.scalar.activation(lss[:], st["ss8"][:], AF.Ln)
        inv8 = sb.tile([128, n], F32, tag="inv8", name="inv8")
        nc.scalar.activation(inv8[:], lss[:], AF.Exp, scale=-0.5)
        t64 = sb.tile([128, n, K], F32, tag="t64", name="t64")
        nc.gpsimd.tensor_tensor(
            out=t64[:], in0=st["rb"][:, :, D:D + K],
            in1=inv8[:].unsqueeze(2).broadcast_to([128, n, K]), op=OP.mult)
        # n = ss * inv = ||y||, into rhs column 64 (after its raw2 read,
        # same in-order Pool queue)
        nc.gpsimd.tensor_tensor(out=st["rb"][:, :, D:D + 1],
                                in0=st["ss8"][:].unsqueeze(2),
                                in1=inv8[:].unsqueeze(2), op=OP.mult)
        st["inv8"], st["t64"] = inv8, t64
        del st["ss8"]

    def a3_exp(st):
        # softmax numerators e = exp(raw2 * inv), then the exp(b_lin)
        # weighting on Pool in-step (am follows E64 in emission order)
        n = st["w"] * SUB
        e64 = sb.tile([128, n, K], F32, tag="e64", name="e64")
        nc.scalar.activation(e64[:], st["t64"][:], AF.Exp)
        am = sb.tile([128, n, K], F32, tag="am", name="am")
        nc.gpsimd.tensor_tensor(out=am[:], in0=e64[:], in1=eblbc[:, :n, :],
                                op=OP.mult)
        st["am"] = am
        del st["t64"]

    def a4_rsum(st):
        # r = 1/sum_k(e * exp(b_lin))
        n = st["w"] * SUB
        rs8 = sb.tile([128, n], F32, tag="rs8", name="rs8")
        nc.vector.reduce_sum(rs8[:], st["am"][:], axis=mybir.AxisListType.X)
        rr8 = sb.tile([128, n], F32, tag="rr8", name="rr8")
        nc.vector.reciprocal(rr8[:], rs8[:])
        st["rr8"] = rr8

    def a4_atil(st):
        # atil = e * (inv * r)
        n = st["w"] * SUB
        q8 = sb.tile([128, n], F32, tag="q8", name="q8")
        nc.gpsimd.tensor_tensor(out=q8[:], in0=st["inv8"][:],
                                in1=st["rr8"][:], op=OP.mult)
        # atil = am * q = e * exp(b_lin) * inv * r, so vlad rows arrive
        # pre-scaled by exp(b_lin) and the finalize skips that multiply
        atil = sb.tile([128, n, K], BF16, tag="atil", name="atil")
        nc.gpsimd.tensor_tensor(
            out=atil[:], in0=st["am"][:],
            in1=q8[:].unsqueeze(2).broadcast_to([128, n, K]), op=OP.mult)
        st["atil"] = atil
        del st["am"], st["inv8"], st["rr8"]

    def a5_agg(st):
        # vladpart[k, :] = sum_s atil_s.T @ [y | n]; one PSUM group per
        # superstep, folded into the batch's SBUF accumulator right after
        n = st["w"] * SUB
        atil, rb = st["atil"], st["rb"]
        vp = vp_pool.tile([K, D + 1], F32, tag="vp", name="vp")
        for s in range(n):
            nc.tensor.matmul(vp[:], atil[:, s, :], rb[:, s, :D + 1],
                             start=(s == 0), stop=(s == n - 1))
        if st["first"]:
            nc.vector.tensor_copy(st["vacc"][:], vp[:])
        else:
            nc.vector.tensor_tensor(out=st["vacc"][:], in0=st["vacc"][:],
                                    in1=vp[:], op=OP.add)
        if st["last"]:
            emit_finalize(st["vacc"], st["b"])
        st.clear()

    def emit_finalize(vlad, b):  # vlad: SBUF accumulator, pre-scaled by ebl
        # finalize batch: centroid subtract, intra-normalize, global norm.
        # The elementwise steps run on Pool so the serial per-batch chain
        # does not stall DVE's pipelined stream.
        cv = sb.tile([K, D], F32, tag="cv", name="cv")
        nc.vector.tensor_scalar_mul(cv[:], cent[:], vlad[:, D:D + 1])
        v = sb.tile([K, D], F32, tag="v", name="v")
        nc.vector.tensor_sub(v[:], vlad[:, :D], cv[:])
        sck = sb.tile([K, D], F32, tag="sck", name="sck")
        nc.vector.tensor_tensor(out=sck[:], in0=v[:], in1=v[:], op=OP.mult)
        ssk = sb.tile([K, 1], F32, tag="ssk", name="ssk")
        nc.vector.reduce_sum(ssk[:], sck[:], axis=mybir.AxisListType.X)
        lk = sb.tile([K, 1], F32, tag="lk", name="lk")
        nc.scalar.activation(lk[:], ssk[:], AF.Ln)
        invk = sb.tile([K, 1], F32, tag="invk", name="invk")
        nc.scalar.activation(invk[:], lk[:], AF.Exp, scale=-0.5)
        # after intra-normalization each of the K rows has norm exactly 1,
        # so the global norm is sqrt(K) (fp error ~1e-7, far under budget);
        # fold 1/sqrt(K) into the intra-norm multiply
        nc.vector.tensor_scalar(
            out=outsb[:, b, :], in0=v[:], scalar1=invk[:],
            scalar2=float(1.0 / np.sqrt(K)), op0=OP.mult, op1=OP.mult)

    # xt DMA groups (in m-tiles): small leading groups fill the pipeline,
    # then GROUP-sized copies amortize the per-copy queue-hold overhead.
    # supersteps: (batch, [tile indices]) with 1-tile steps early on.
    groups = []       # (batch, m0_tiles, n_tiles)
    tile_of = {}      # global tile idx -> (group, offset)
    supers = []       # (batch, [global tile idx], first, last)
    def chunks(total, first, size):
        out = list(first)
        left = total - sum(first)
        while left > 0:
            c = min(size, left)
            out.append(c)
            left -= c
        return out

    gsizes0 = chunks(N_TILES, [1, 1], GROUP)
    ssizes0 = [1, 1, 2, 2, 2]
    gsizes = chunks(N_TILES, [], GROUP)
    ssizes = [2] * (N_TILES // 2)
    ssizesN = [2] * (N_TILES // 2)
    gtile = 0
    for b in range(B_LOC):
        t0 = 0
        for gsz in (gsizes0 if b == 0 else gsizes):
            g = len(groups)
            groups.append((b, t0, gsz))
            for j in range(gsz):
                tile_of[gtile + t0 + j] = (g, j)
            t0 += gsz
        assert t0 == N_TILES
        t0 = 0
        ss = ssizes0 if b == 0 else (ssizesN if b == B_LOC - 1 else ssizes)
        for ssz in ss:
            supers.append((b, [gtile + t0 + j for j in range(ssz)],
                           t0 == 0, t0 + ssz == N_TILES))
            t0 += ssz
        assert t0 == N_TILES
        gtile += N_TILES

    xt_bs = [xt_d[b].rearrange("(j p) m -> p j m", p=128) for b in range(B_LOC)]
    xtg = {}

    def emit_group(g):
        gb, gt0, gn = groups[g]
        xtg[g] = emit_dma(xt_bs[gb], gt0 * M_TILE, gn * M_TILE)
        if g == 3:
            emit_consts_mid()
        elif g == 5:
            emit_consts_late()

    def xt_slice(ti):
        g, off = tile_of[ti]
        return xtg[g][:, :, off * M_TILE:(off + 1) * M_TILE]

    emit_consts_early()
    lead0 = 0
    t_acc = 0
    while lead0 < len(groups) and t_acc < PREFETCH_T:
        t_acc += groups[lead0][2]
        lead0 += 1
    for g in range(lead0):
        emit_group(g)
    next_g = [lead0]
    states = {}
    vaccs = {}
    n_sup = len(supers)
    emitted_tiles = 0
    for i in range(n_sup + DEFER + 1):  # DEFER == pipeline depth (5)
        if i < n_sup:
            b, tids, first, last = supers[i]
            if first:
                vaccs[b] = vaccp.tile([K, D + 1], F32, tag="vacc",
                                      name="vacc")
            # keep the DMA queues PREFETCH_G groups ahead of consumption
            emitted_tiles += len(tids)
            while next_g[0] < len(groups) and \
                    sum(groups[g][2] for g in range(next_g[0])) < \
                    emitted_tiles + PREFETCH_T:
                emit_group(next_g[0])
                next_g[0] += 1
            states[i] = {"w": len(tids), "b": b, "first": first,
                         "last": last, "vacc": vaccs[b],
                         "xts": [xt_slice(t) for t in tids]}
        # per-engine emission order tuned so no in-order queue head-blocks:
        #   PE   mm(i), agg(i-5)
        #   ACT  E64(i-3), ycopy(i), Ln/Exp(i-2)
        #   DVE  rs8/rcp(i-4), sq/red(i-1), vacc(i-5)
        #   Pool am(i-3), t64/n(i-2), q8/atil(i-4)
        if i < n_sup:
            a0_matmul(states[i])
        if 0 <= i - 3 < n_sup:
            a3_exp(states[i - 3])
        if i < n_sup:
            a0_ycopy(states[i])
        if 0 <= i - 2 < n_sup:
            a2_inv(states[i - 2])
        if 0 <= i - 4 < n_sup:
            a4_rsum(states[i - 4])
        if 0 <= i - 1 < n_sup:
            a1_norm2(states[i - 1])
        if 0 <= i - 4 < n_sup:
            a4_atil(states[i - 4])
        if i - 5 >= 0 and states.get(i - 5):
            a5_agg(states.pop(i - 5))

    nc.sync.dma_start(out_d.rearrange("b (k d) -> k b d", k=K), outsb[:])



_CACHE = {}


def _patch_act_tables():
    """Force all Exp/Ln/Square activations to resolve in the one table set
    that contains them all (natural_log_exp_and_others), so bacc's
    insert_act_table_loads emits a single hoisted LoadActFuncSet instead of
    thrashing between exp_and_others and natural_log per tile (~2.7us per
    reload).  List order/length is preserved so act_func_set_id stays a
    valid index into act_info.json."""
    import concourse.bacc as bacc_mod
    import concourse.hw_specs as hw_specs
    if _CACHE.get("act_patched"):
        return
    orig = hw_specs.get_activation_tables
    AF = mybir.ActivationFunctionType
    strip = {AF.Exp, AF.Ln, AF.Square}
    keep = "natural_log_exp_and_others"

    def patched(arch):
        tables = orig(arch)
        return {
            name: (set(fns) if name == keep else set(fns) - strip)
            for name, fns in tables.items()
        }

    bacc_mod.get_activation_tables = patched
    _CACHE["act_patched"] = True


def _declare_io(nc):
    xt_d = nc.dram_tensor("xt", [B_LOC, C, M], FP8,
                          kind="ExternalInput").ap()
    wcat_d = nc.dram_tensor("wcat", [NCH, 128, 72], FP8,
                            kind="ExternalInput").ap()
    bcat_d = nc.dram_tensor("bcat", [1, 72], FP8,
                            kind="ExternalInput").ap()
    eblbc_d = nc.dram_tensor("eblbc", [128, 2 * SUB, K], F32,
                             kind="ExternalInput").ap()
    ebl8_d = nc.dram_tensor("ebl8", [K, 1], F32, kind="ExternalInput").ap()
    cent_d = nc.dram_tensor("cent", [K, D], F32, kind="ExternalInput").ap()
    out_d = nc.dram_tensor("out", [B_LOC, K * D], F32, kind="ExternalOutput").ap()
    return out_d, xt_d, wcat_d, bcat_d, eblbc_d, ebl8_d, cent_d


def _build_program():
    if "nc" in _CACHE:
        return _CACHE["nc"]
    _patch_act_tables()
    nc = bacc.Bacc("TRN2", target_bir_lowering=False, debug=False,
                   num_devices=N_CORES)
    out_d, xt_d, wcat_d, bcat_d, eblbc_d, ebl8_d, cent_d = _declare_io(nc)

    with tile.TileContext(nc) as tc:
        _netvlad_kernel(tc, out_d, xt_d, wcat_d, bcat_d, eblbc_d, ebl8_d, cent_d)
    nc.compile()
    _CACHE["nc"] = nc
    return nc


def _prep_inputs(x, W_red, b_red, W_lin, b_lin, centroids):
    wcat = np.concatenate([W_red.T, W_red.T @ W_lin.T], axis=1)     # [512, 72]
    wcat = np.ascontiguousarray(wcat.astype(fp8).reshape(NCH, 128, 72))
    bcat = np.concatenate([b_red, W_lin @ b_red]).astype(fp8)[None, :]
    ebl = np.exp(b_lin).astype(np.float32)
    eblbc = np.ascontiguousarray(
        np.broadcast_to(ebl, (128, 2 * SUB, K)).astype(np.float32))
    ebl8 = ebl[:, None]
    cent = centroids.astype(np.float32)
    xt = np.ascontiguousarray(x.astype(fp8).transpose(0, 2, 1))     # [B, C, M]
    return xt, wcat, bcat, eblbc, ebl8, cent


def kernel(x, mask, W_red, b_red, W_lin, b_lin, centroids, **kwargs):
    x = np.asarray(x, dtype=np.float32)
    W_red = np.asarray(W_red, dtype=np.float32)
    b_red = np.asarray(b_red, dtype=np.float32)
    W_lin = np.asarray(W_lin, dtype=np.float32)
    b_lin = np.asarray(b_lin, dtype=np.float32)
    centroids = np.asarray(centroids, dtype=np.float32)

    xt, wcat, bcat, eblbc, ebl8, cent = _prep_inputs(
        x, W_red, b_red, W_lin, b_lin, centroids)

    nc = _build_program()
    in_maps = []
    for i in range(N_CORES):
        in_maps.append({
            "xt": np.ascontiguousarray(xt[i * B_LOC:(i + 1) * B_LOC]),
            "wcat": wcat, "bcat": bcat, "eblbc": eblbc,
            "ebl8": ebl8, "cent": cent,
        })
    res = run_bass_kernel_spmd(nc, in_maps, list(range(N_CORES)),
                               **kwargs.get("_run_kwargs", {}))
    out = np.concatenate([res.results[i]["out"] for i in range(N_CORES)], axis=0)
    if kwargs.get("_return_raw"):
        return out, res
    return out


# revision 54
# speedup vs baseline: 1.0247x; 1.0040x over previous
"""NetVLAD pooling kernel for Trainium2, data-parallel over batch across 8 cores.

Computation per batch b (reference semantics):
  y      = x @ W_red.T + b_red            # [m, 64]
  yn     = y / ||y||_row                  # L2 normalize rows
  logits = yn @ W_lin.T + b_lin           # [m, 8]
  a      = softmax(logits, axis=1)
  vlad   = a.T @ yn - centroids * a.sum(0)[:, None]
  out    = l2norm_global(l2norm_rows(vlad).flatten())

Device-side algebra (per row m):
  yz   = x @ [W_red.T | W_red.T W_lin.T] + [b_red | W_lin b_red]   # fused [m, 72]
  inv  = exp(-0.5 ln(sum(y^2)))        # 1/||y|| via the ln/exp table set
  n    = ss * inv                      # ||y||
  e    = exp(raw2 * inv)               # un-biased softmax numerator
  r    = 1 / sum_k(e * exp(b_lin))
  atil = e * exp(b_lin) * (inv * r)    # atil.T @ [y | n] = ebl*[a'.T yn | a'.sum]
  so the per-batch accumulator arrives pre-scaled by exp(b_lin) and the
  finalize is just centroid-subtract + intra/global normalization.

Layout and schedule:
- x ships pre-transposed to [b, C, m] in fp8e4m3: the contraction dim lands
  on SBUF partitions with 1KB-contiguous DMA descriptors at 1 byte/elem
  (the kernel is HBM-bound: 16.8 MB/core floor).
- The reduction matmul runs in fp8 DoubleRow mode: two 128-row contraction
  chunks per instruction at half per-row cost (4x fewer PE cycles than
  four plain chunk matmuls).
- Work is organized in 2048-row supersteps, software-pipelined in 6 stages
  (mm+ycopy | square+reduce | ln/exp + t64/n | exp + ebl | rsum | atil) with
  the aggregation matmuls 5 supersteps behind, so every cross-engine
  dependency is about a superstep old when its in-order consumer reaches it.
- Engine placement keeps each engine under the 2.9us/superstep DMA pace:
  ACT does the PSUM->SBUF bf16 y copy (GPSIMD cannot touch PSUM) plus
  Ln/Exp/softmax-Exp; DVE does the square (2x bf16 rate), the big reduce
  and r; Pool (GpSimd) does the inv scalings and exp(b_lin) weighting.
- Each superstep's aggregation closes its own PSUM group, folded into an
  SBUF accumulator, so no PSUM bank is held across the deferral window
  (PSUM: 3x2 yz banks + 2 vladpart banks = 8).
- xt loads go out in 2-tile groups, rotated across the SP and ACT hardware
  DGE queues ~14 tiles ahead of consumption (the issuing sequencer is held
  for descriptor-gen plus wire wait, so one queue alone cannot keep the
  wire saturated).
- The output is centroid-dominated (||a.T yn|| ~ 21 vs ||cent * asum|| ~
  4700), so fp8 x / bf16 y quantization lands ~3e-4 relative to output
  scale, far under the 2e-2 gate.
"""
import numpy as np
import ml_dtypes
from contextlib import ExitStack

import concourse.bass as bass
import concourse.tile as tile
import concourse.bass_isa as bass_isa
from concourse import bacc, mybir
from concourse._compat import with_exitstack
from concourse.bass_utils import run_bass_kernel_spmd

bf16 = ml_dtypes.bfloat16
F32 = mybir.dt.float32
BF16 = mybir.dt.bfloat16
FP8 = mybir.dt.float8e4
fp8 = ml_dtypes.float8_e4m3

# tuning toggles (read at program-build time)
SB_BUFS = 8
XT_BUFS = 9          # xt group buffers (each up to GROUP m-tiles)
DEFER = 5            # supersteps between a superstep's matmul and its agg
PREFETCH_T = 14      # m-tiles of xt DMA lookahead
GROUP = 2            # m-tiles per xt DMA (amortizes per-copy queue overhead)
DMA_QUEUES = ("sync", "scalar")   # rotate xt groups across HWDGE queues

N_CORES = 8
B, M, C = 32, 8192, 512
K, D = 8, 64
B_LOC = B // N_CORES          # 4 batches per core
M_TILE = 1024
N_TILES = M // M_TILE         # 8
SUB = M_TILE // 128           # 8 subtiles of 128 rows
NCH = C // 128                # 4 contraction chunks


@with_exitstack
def _netvlad_kernel(ctx: ExitStack, tc: tile.TileContext, out_d, xt_d, wcat_d,
                    bcat_d, eblbc_d, ebl8_d, cent_d):
    nc = tc.nc
    AF = mybir.ActivationFunctionType
    OP = mybir.AluOpType
    PM = mybir.MatmulPerfMode

    consts = ctx.enter_context(tc.tile_pool(name="consts", bufs=1))
    xt_pool = ctx.enter_context(tc.tile_pool(name="xt", bufs=XT_BUFS))
    sb = ctx.enter_context(tc.tile_pool(name="work", bufs=SB_BUFS))
    vaccp = ctx.enter_context(tc.tile_pool(name="vaccp", bufs=2))
    outp = ctx.enter_context(tc.tile_pool(name="outp", bufs=1))
    yz_pool = ctx.enter_context(tc.tile_pool(name="yz", bufs=3, space="PSUM"))
    vp_pool = ctx.enter_context(tc.tile_pool(name="vp", bufs=2, space="PSUM"))

    # constants: wcat/bcat are needed by the very first matmuls; the rest
    # are deferred into the xt group stream (emitted by the prefetch pump)
    # so their per-copy queue overhead doesn't delay the first x tiles
    wcat = consts.tile([128, NCH, 72], FP8)
    bcat = consts.tile([1, 72], FP8)
    eblbc = consts.tile([128, 2 * SUB, K], F32)
    cent = consts.tile([K, D], F32)
    ones = consts.tile([1, 128], FP8)
    nc.vector.memset(ones[:], 1.0)

    def emit_consts_early():
        nc.sync.dma_start(wcat[:], wcat_d.rearrange("j p t -> p j t"))
        nc.sync.dma_start(bcat[:], bcat_d[:])

    def emit_consts_mid():
        nc.scalar.dma_start(eblbc[:], eblbc_d[:])

    def emit_consts_late():
        nc.scalar.dma_start(cent[:], cent_d[:])

    outsb = outp.tile([K, B_LOC, D], F32)

    dma_idx = [1]   # first group on the scalar queue, parallel to wcat

    def emit_dma(xt_b, m0, rows):
        """Prefetch a group of m-tiles of x on a rotating HWDGE queue."""
        xt = xt_pool.tile([128, NCH, rows], FP8, tag="xt", name="xt")
        q = getattr(nc, DMA_QUEUES[dma_idx[0] % len(DMA_QUEUES)])
        dma_idx[0] += 1
        q.dma_start(xt[:], xt_b[:, :, m0:m0 + rows])
        return xt

    # Each superstep covers up to two 1024-row m-tiles (2048 descriptors) so
    # per-instruction overheads amortize over twice the data.  The chain is
    # software-pipelined: stage Ak of superstep i is emitted at step i+k, so
    # every cross-engine dependency is a superstep (~2.9us of work) old when
    # the consumer's in-order queue reaches it.  Per-superstep engine
    # budgets, against the ~2.9us DMA wire pace:
    #   ACT  2x ycopy 665 + Ln 198 + Exp 198 + E64 292   ~ 2.0us
    #   DVE  sq 593 + red 1127 + rs8 193 + rcp 77
    #        + q8 77 + atil 254 + vacc 193               ~ 2.5us
    #   Pool t64 444 + n 250 + am 349                    ~ 1.0us
    #   PE   48 mm + 16 agg mm                           ~ 1.4us
    # The y copy carries all 72 yz columns to SBUF in bf16 (y | raw2); the
    # squares come from its bf16 y at the DVE 2x rate, raw2 is consumed in
    # bf16, and column 64 is overwritten with n = ||y|| (same in-order Pool
    # queue as its reader) to form the agg rhs [y | n].  Each superstep's
    # agg matmuls close their own PSUM accumulation group (vladpart) which
    # DVE folds into an SBUF accumulator, so no PSUM bank is held across the
    # deferral window.  GPSIMD cannot touch PSUM, so PSUM reads sit on
    # ACT/DVE only.
    def a0_matmul(st):
        # fused reduction+logits matmul: yz[m, :72] = x @ Wcat + bcat
        # (fp8 DoubleRow: two 128-row chunks per instruction, half per-row
        # cost)
        W = st["w"]
        yzs = []
        for t in range(W):
            xt = st["xts"][t]
            yz = yz_pool.tile([128, SUB, 128], F32, tag="yz", name="yz")
            for s in range(SUB):
                sc = slice(s * 128, (s + 1) * 128)
                nc.tensor.matmul(yz[:, s, :72], xt[:, 0:2, sc],
                                 wcat[:, 0:2, :], start=True, stop=False,
                                 perf_mode=PM.DoubleRow)
                nc.tensor.matmul(yz[:, s, :72], xt[:, 2:4, sc],
                                 wcat[:, 2:4, :], start=False, stop=False,
                                 perf_mode=PM.DoubleRow)
                nc.tensor.matmul(yz[:, s, :72], ones[:], bcat[:],
                                 start=False, stop=True)
            yzs.append(yz)
        st["yzs"] = yzs
        del st["xts"]

    def a0_ycopy(st):
        # PSUM -> SBUF in bf16 on ACT (GPSIMD cannot touch PSUM)
        W = st["w"]
        rb = sb.tile([128, W * SUB, 72], BF16, tag="rb", name="rb")
        for t, yz in enumerate(st["yzs"]):
            nc.scalar.activation(rb[:, t * SUB:(t + 1) * SUB, :],
                                 yz[:, :, :72], AF.Copy)
        st["rb"] = rb
        del st["yzs"]

    def a1_norm2(st):
        # ss = sum(y^2) per row, from the bf16 y at the DVE 2x rate
        n = st["w"] * SUB
        sqs = sb.tile([128, n, D], BF16, tag="sqs", name="sqs")
        nc.vector.tensor_tensor(out=sqs[:], in0=st["rb"][:, :, :D],
                                in1=st["rb"][:, :, :D], op=OP.mult)
        ss8 = sb.tile([128, n], F32, tag="ss8", name="ss8")
        nc.vector.reduce_sum(ss8[:], sqs[:], axis=mybir.AxisListType.X)
        st["ss8"] = ss8

    def a2_inv(st):
        # inv = 1/||y|| via the ln/exp table set; Pool consumes it in-step
        # (its queue reaches t64 after this ACT pair has retired)
        n = st["w"] * SUB
        lss = sb.tile([128, n], F32, tag="lss", name="lss")
        nc.scalar.activation(lss[:], st["ss8"][:], AF.Ln)
        inv8 = sb.tile([128, n], F32, tag="inv8", name="inv8")
        nc.scalar.activation(inv8[:], lss[:], AF.Exp, scale=-0.5)
        t64 = sb.tile([128, n, K], F32, tag="t64", name="t64")
        nc.gpsimd.tensor_tensor(
            out=t64[:], in0=st["rb"][:, :, D:D + K],
            in1=inv8[:].unsqueeze(2).broadcast_to([128, n, K]), op=OP.mult)
        # n = ss * inv = ||y||, into rhs column 64 (after its raw2 read,
        # same in-order Pool queue)
        nc.gpsimd.tensor_tensor(out=st["rb"][:, :, D:D + 1],
                                in0=st["ss8"][:].unsqueeze(2),
                                in1=inv8[:].unsqueeze(2), op=OP.mult)
        st["inv8"], st["t64"] = inv8, t64
        del st["ss8"]

    def a3_exp(st):
        # softmax numerators e = exp(raw2 * inv), then the exp(b_lin)
        # weighting on Pool in-step (am follows E64 in emission order)
        n = st["w"] * SUB
        e64 = sb.tile([128, n, K], F32, tag="e64", name="e64")
        nc.scalar.activation(e64[:], st["t64"][:], AF.Exp)
        am = sb.tile([128, n, K], F32, tag="am", name="am")
        nc.gpsimd.tensor_tensor(out=am[:], in0=e64[:], in1=eblbc[:, :n, :],
                                op=OP.mult)
        st["am"] = am
        del st["t64"]

    def a4_rsum(st):
        # r = 1/sum_k(e * exp(b_lin))
        n = st["w"] * SUB
        rs8 = sb.tile([128, n], F32, tag="rs8", name="rs8")
        nc.vector.reduce_sum(rs8[:], st["am"][:], axis=mybir.AxisListType.X)
        rr8 = sb.tile([128, n], F32, tag="rr8", name="rr8")
        nc.vector.reciprocal(rr8[:], rs8[:])
        st["rr8"] = rr8

    def a4_atil(st):
        # atil = e * (inv * r)
        n = st["w"] * SUB
        q8 = sb.tile([128, n], F32, tag="q8", name="q8")
        nc.gpsimd.tensor_tensor(out=q8[:], in0=st["inv8"][:],
                                in1=st["rr8"][:], op=OP.mult)
        # atil = am * q = e * exp(b_lin) * inv * r, so vlad rows arrive
        # pre-scaled by exp(b_lin) and the finalize skips that multiply
        atil = sb.tile([128, n, K], BF16, tag="atil", name="atil")
        nc.gpsimd.tensor_tensor(
            out=atil[:], in0=st["am"][:],
            in1=q8[:].unsqueeze(2).broadcast_to([128, n, K]), op=OP.mult)
        st["atil"] = atil
        del st["am"], st["inv8"], st["rr8"]

    def a5_agg(st):
        # vladpart[k, :] = sum_s atil_s.T @ [y | n]; one PSUM group per
        # superstep, folded into the batch's SBUF accumulator right after
        n = st["w"] * SUB
        atil, rb = st["atil"], st["rb"]
        vp = vp_pool.tile([K, D + 1], F32, tag="vp", name="vp")
        for s in range(n):
            nc.tensor.matmul(vp[:], atil[:, s, :], rb[:, s, :D + 1],
                             start=(s == 0), stop=(s == n - 1))
        if st["first"]:
            nc.vector.tensor_copy(st["vacc"][:], vp[:])
        else:
            nc.vector.tensor_tensor(out=st["vacc"][:], in0=st["vacc"][:],
                                    in1=vp[:], op=OP.add)
        if st["last"]:
            emit_finalize(st["vacc"], st["b"])
        st.clear()

    def emit_finalize(vlad, b):  # vlad: SBUF accumulator, pre-scaled by ebl
        # finalize batch: centroid subtract, intra-normalize, global norm
        cv = sb.tile([K, D], F32, tag="cv", name="cv")
        nc.vector.tensor_scalar_mul(cv[:], cent[:], vlad[:, D:D + 1])
        v = sb.tile([K, D], F32, tag="v", name="v")
        nc.vector.tensor_sub(v[:], vlad[:, :D], cv[:])
        sck = sb.tile([K, D], F32, tag="sck", name="sck")
        nc.vector.tensor_tensor(out=sck[:], in0=v[:], in1=v[:], op=OP.mult)
        ssk = sb.tile([K, 1], F32, tag="ssk", name="ssk")
        nc.vector.reduce_sum(ssk[:], sck[:], axis=mybir.AxisListType.X)
        lk = sb.tile([K, 1], F32, tag="lk", name="lk")
        nc.scalar.activation(lk[:], ssk[:], AF.Ln)
        invk = sb.tile([K, 1], F32, tag="invk", name="invk")
        nc.scalar.activation(invk[:], lk[:], AF.Exp, scale=-0.5)
        # after intra-normalization each of the K rows has norm exactly 1,
        # so the global norm is sqrt(K) (fp error ~1e-7, far under budget);
        # fold 1/sqrt(K) into the intra-norm multiply
        nc.vector.tensor_scalar(
            out=outsb[:, b, :], in0=v[:], scalar1=invk[:],
            scalar2=float(1.0 / np.sqrt(K)), op0=OP.mult, op1=OP.mult)

    # xt DMA groups (in m-tiles): small leading groups fill the pipeline,
    # then GROUP-sized copies amortize the per-copy queue-hold overhead.
    # supersteps: (batch, [tile indices]) with 1-tile steps early on.
    groups = []       # (batch, m0_tiles, n_tiles)
    tile_of = {}      # global tile idx -> (group, offset)
    supers = []       # (batch, [global tile idx], first, last)
    def chunks(total, first, size):
        out = list(first)
        left = total - sum(first)
        while left > 0:
            c = min(size, left)
            out.append(c)
            left -= c
        return out

    gsizes0 = chunks(N_TILES, [1, 1, 1, 1], GROUP)
    ssizes0 = [1, 1, 1, 1, 2, 2]
    gsizes = chunks(N_TILES, [], GROUP)
    ssizes = [2] * (N_TILES // 2)
    ssizesN = [2] * (N_TILES // 2)
    gtile = 0
    for b in range(B_LOC):
        t0 = 0
        for gsz in (gsizes0 if b == 0 else gsizes):
            g = len(groups)
            groups.append((b, t0, gsz))
            for j in range(gsz):
                tile_of[gtile + t0 + j] = (g, j)
            t0 += gsz
        assert t0 == N_TILES
        t0 = 0
        ss = ssizes0 if b == 0 else (ssizesN if b == B_LOC - 1 else ssizes)
        for ssz in ss:
            supers.append((b, [gtile + t0 + j for j in range(ssz)],
                           t0 == 0, t0 + ssz == N_TILES))
            t0 += ssz
        assert t0 == N_TILES
        gtile += N_TILES

    xt_bs = [xt_d[b].rearrange("(j p) m -> p j m", p=128) for b in range(B_LOC)]
    xtg = {}

    def emit_group(g):
        gb, gt0, gn = groups[g]
        xtg[g] = emit_dma(xt_bs[gb], gt0 * M_TILE, gn * M_TILE)
        if g == 3:
            emit_consts_mid()
        elif g == 5:
            emit_consts_late()

    def xt_slice(ti):
        g, off = tile_of[ti]
        return xtg[g][:, :, off * M_TILE:(off + 1) * M_TILE]

    emit_consts_early()
    lead0 = 0
    t_acc = 0
    while lead0 < len(groups) and t_acc < PREFETCH_T:
        t_acc += groups[lead0][2]
        lead0 += 1
    for g in range(lead0):
        emit_group(g)
    next_g = [lead0]
    states = {}
    vaccs = {}
    n_sup = len(supers)
    emitted_tiles = 0
    for i in range(n_sup + DEFER + 1):  # DEFER == pipeline depth (5)
        if i < n_sup:
            b, tids, first, last = supers[i]
            if first:
                vaccs[b] = vaccp.tile([K, D + 1], F32, tag="vacc",
                                      name="vacc")
            # keep the DMA queues PREFETCH_G groups ahead of consumption
            emitted_tiles += len(tids)
            while next_g[0] < len(groups) and \
                    sum(groups[g][2] for g in range(next_g[0])) < \
                    emitted_tiles + PREFETCH_T:
                emit_group(next_g[0])
                next_g[0] += 1
            states[i] = {"w": len(tids), "b": b, "first": first,
                         "last": last, "vacc": vaccs[b],
                         "xts": [xt_slice(t) for t in tids]}
        # per-engine emission order tuned so no in-order queue head-blocks:
        #   PE   mm(i), agg(i-5)
        #   ACT  E64(i-3), ycopy(i), Ln/Exp(i-2)
        #   DVE  rs8/rcp(i-4), sq/red(i-1), vacc(i-5)
        #   Pool am(i-3), t64/n(i-2), q8/atil(i-4)
        if i < n_sup:
            a0_matmul(states[i])
        if 0 <= i - 3 < n_sup:
            a3_exp(states[i - 3])
        if i < n_sup:
            a0_ycopy(states[i])
        if 0 <= i - 2 < n_sup:
            a2_inv(states[i - 2])
        if 0 <= i - 4 < n_sup:
            a4_rsum(states[i - 4])
        if 0 <= i - 1 < n_sup:
            a1_norm2(states[i - 1])
        if 0 <= i - 4 < n_sup:
            a4_atil(states[i - 4])
        if i - 5 >= 0 and states.get(i - 5):
            a5_agg(states.pop(i - 5))

    nc.sync.dma_start(out_d.rearrange("b (k d) -> k b d", k=K), outsb[:])



_CACHE = {}


def _patch_act_tables():
    """Force all Exp/Ln/Square activations to resolve in the one table set
    that contains them all (natural_log_exp_and_others), so bacc's
    insert_act_table_loads emits a single hoisted LoadActFuncSet instead of
    thrashing between exp_and_others and natural_log per tile (~2.7us per
    reload).  List order/length is preserved so act_func_set_id stays a
    valid index into act_info.json."""
    import concourse.bacc as bacc_mod
    import concourse.hw_specs as hw_specs
    if _CACHE.get("act_patched"):
        return
    orig = hw_specs.get_activation_tables
    AF = mybir.ActivationFunctionType
    strip = {AF.Exp, AF.Ln, AF.Square}
    keep = "natural_log_exp_and_others"

    def patched(arch):
        tables = orig(arch)
        return {
            name: (set(fns) if name == keep else set(fns) - strip)
            for name, fns in tables.items()
        }

    bacc_mod.get_activation_tables = patched
    _CACHE["act_patched"] = True


def _declare_io(nc):
    xt_d = nc.dram_tensor("xt", [B_LOC, C, M], FP8,
                          kind="ExternalInput").ap()
    wcat_d = nc.dram_tensor("wcat", [NCH, 128, 72], FP8,
                            kind="ExternalInput").ap()
    bcat_d = nc.dram_tensor("bcat", [1, 72], FP8,
                            kind="ExternalInput").ap()
    eblbc_d = nc.dram_tensor("eblbc", [128, 2 * SUB, K], F32,
                             kind="ExternalInput").ap()
    ebl8_d = nc.dram_tensor("ebl8", [K, 1], F32, kind="ExternalInput").ap()
    cent_d = nc.dram_tensor("cent", [K, D], F32, kind="ExternalInput").ap()
    out_d = nc.dram_tensor("out", [B_LOC, K * D], F32, kind="ExternalOutput").ap()
    return out_d, xt_d, wcat_d, bcat_d, eblbc_d, ebl8_d, cent_d


def _build_program():
    if "nc" in _CACHE:
        return _CACHE["nc"]
    _patch_act_tables()
    nc = bacc.Bacc("TRN2", target_bir_lowering=False, debug=False,
                   num_devices=N_CORES)
    out_d, xt_d, wcat_d, bcat_d, eblbc_d, ebl8_d, cent_d = _declare_io(nc)

    with tile.TileContext(nc) as tc:
        _netvlad_kernel(tc, out_d, xt_d, wcat_d, bcat_d, eblbc_d, ebl8_d, cent_d)
    nc.compile()
    _CACHE["nc"] = nc
    return nc


def _prep_inputs(x, W_red, b_red, W_lin, b_lin, centroids):
    wcat = np.concatenate([W_red.T, W_red.T @ W_lin.T], axis=1)     # [512, 72]
    wcat = np.ascontiguousarray(wcat.astype(fp8).reshape(NCH, 128, 72))
    bcat = np.concatenate([b_red, W_lin @ b_red]).astype(fp8)[None, :]
    ebl = np.exp(b_lin).astype(np.float32)
    eblbc = np.ascontiguousarray(
        np.broadcast_to(ebl, (128, 2 * SUB, K)).astype(np.float32))
    ebl8 = ebl[:, None]
    cent = centroids.astype(np.float32)
    xt = np.ascontiguousarray(x.astype(fp8).transpose(0, 2, 1))     # [B, C, M]
    return xt, wcat, bcat, eblbc, ebl8, cent


def kernel(x, mask, W_red, b_red, W_lin, b_lin, centroids, **kwargs):
    x = np.asarray(x, dtype=np.float32)
    W_red = np.asarray(W_red, dtype=np.float32)
    b_red = np.asarray(b_red, dtype=np.float32)
    W_lin = np.asarray(W_lin, dtype=np.float32)
    b_lin = np.asarray(b_lin, dtype=np.float32)
    centroids = np.asarray(centroids, dtype=np.float32)

    xt, wcat, bcat, eblbc, ebl8, cent = _prep_inputs(
        x, W_red, b_red, W_lin, b_lin, centroids)

    nc = _build_program()
    in_maps = []
    for i in range(N_CORES):
        in_maps.append({
            "xt": np.ascontiguousarray(xt[i * B_LOC:(i + 1) * B_LOC]),
            "wcat": wcat, "bcat": bcat, "eblbc": eblbc,
            "ebl8": ebl8, "cent": cent,
        })
    res = run_bass_kernel_spmd(nc, in_maps, list(range(N_CORES)),
                               **kwargs.get("_run_kwargs", {}))
    out = np.concatenate([res.results[i]["out"] for i in range(N_CORES)], axis=0)
    if kwargs.get("_return_raw"):
        return out, res
    return out
